# revision 17
# baseline (speedup 1.0000x reference)
"""nn_Attention: out[b,h] = strict_tril(rope(Q[b,h]) @ rope(Q[b,h])^T) @ V[b].

Sharding: one (b,h) pair per NeuronCore (B*H = 8 pairs on 8 cores, fully
data-parallel, no collectives).

Host-side staging de-interleaves Q's even/odd columns AND transposes it
(both pure relayouts: scores contract over all of n, so any fixed
n-permutation is mathematically neutral, and the transpose just picks
which axis lands on SBUF partitions), plus casts to bf16 (the kernel
cast-loaded to bf16 anyway).  RoPE is then computed DIRECTLY in the
QR^T chunk layout the score matmuls need as both lhsT and rhs - the
PE-transpose phase of the previous design (62us of PE time) disappears.

Per core, in waves of tq=512 t-columns:

  phase 0 : load qT pair-chunk tiles (Qe^T rows 0..n/2, Qo^T rows n/2..)
            and transposed cos/sin tables for the wave's t-range; RoPE on
            DVE with dense step-1 bf16 ops (2x mode):
              QRe_k = Qe_k*c_k - Qo_k*s_k -> qrt chunk k
              QRo_k = Qo_k*c_k + Qe_k*s_k -> qrt chunk 8+k
  phase A : score strips T_j = QR_j @ QR^T[:, lo:wave_end] (upper-triangle
            blocks only; scores are symmetric so T_ji doubles as the
            transposed lhsT for phase B), 512-wide f32 PSUM tiles
            (LDWEIGHTS fully hidden at this width), strict-upper mask on
            the diagonal block, cast to bf16 strips.
  phase B : out_i = sum_{j<=i} matmul(lhsT=T_ji, rhs=V_j) accumulated in
            PSUM, copied out as f32 and stored per row block.
"""

import math
from functools import lru_cache

import numpy as np
import ml_dtypes

import concourse.bass as bass
import concourse.mybir as mybir
import concourse.tile as tile
from concourse import bacc
from concourse.bass_utils import run_bass_kernel_spmd
from concourse.masks import make_upper_triangular

THETA = 2.0 ** 16
P = 128
TMODE = "tr"  # kept for test.py --tmode compat; unused

BF16 = mybir.dt.bfloat16
F32 = mybir.dt.float32


@lru_cache(maxsize=None)
def _rope_tables(t, n):
    """Transposed cos/sin tables matching reference._rope, bf16.

    cosT[p, t] = cos(phase[t, 2p]), sinT[p, t] = sin(phase[t, 2p]);
    one entry per pair (reference quantizes freqs in pairs).
    """
    idx = ((np.arange(n) // 2) * 2).astype(np.float32)
    freqs = (1.0 / (THETA ** (idx / np.float32(n))) / np.float32(2.0 * math.pi)).astype(
        np.float32
    )
    pos = np.arange(t, dtype=np.float32)[:, None]
    phases = ((pos * freqs) % np.float32(1.0)) * np.float32(2.0 * math.pi)
    cos_h = np.cos(phases)[:, 0::2]
    sin_h = np.sin(phases)[:, 0::2]
    return np.ascontiguousarray(
        np.vstack([cos_h.T, sin_h.T]).astype(ml_dtypes.bfloat16)
    )


def _wave_bounds(t):
    """Wave column boundaries: small warmup waves (so the first DMAs land
    and the DVE RoPE for wave w+1 finishes before the PE drains wave w's
    matmuls), then 512-wide steady-state waves."""
    bounds = [0]
    for wsz in (128, 128, 256):
        if bounds[-1] + wsz <= t:
            bounds.append(bounds[-1] + wsz)
    while bounds[-1] < t:
        bounds.append(min(t, bounds[-1] + 512))
    return bounds


@lru_cache(maxsize=None)
def _build(t, n, d):
    from contextlib import ExitStack

    nt = t // P        # row blocks
    nk = n // P        # contraction chunks
    half = nk // 2     # pair chunks
    bounds = _wave_bounds(t)
    assert n % (2 * P) == 0 and t % P == 0

    nc = bacc.Bacc("TRN2", target_bir_lowering=False, debug=False, num_swdge_queues=4)
    qt_d = nc.dram_tensor("qt", [n, t], BF16, kind="ExternalInput").ap()
    v_d = nc.dram_tensor("v", [t, d], BF16, kind="ExternalInput").ap()
    # cos table rows stacked over sin table rows: [n, t]
    cs_d = nc.dram_tensor("cs", [n, t], BF16, kind="ExternalInput").ap()
    out_d = nc.dram_tensor("out", [t, d], F32, kind="ExternalOutput").ap()

    with tile.TileContext(nc) as tc, ExitStack() as ctx:
        const = ctx.enter_context(tc.tile_pool(name="const", bufs=1))
        umask = const.tile([P, P], BF16, name="umask")

        vpool = ctx.enter_context(tc.tile_pool(name="vpool", bufs=1))
        vb = vpool.tile([P, nt * d], BF16, name="vb")
        # head blocks of V, duplicate-loaded early on the ACT ring so the
        # tiny phase B of the warmup waves never waits on the big V load
        nvh = min(nt, 4)
        vh = vpool.tile([P, nvh * d], BF16, name="vh")

        qrt_pool = ctx.enter_context(tc.tile_pool(name="qrt_pool", bufs=1))
        # QR^T: chunk k ([n in [kP,(k+1)P)] x [t]) lives at cols [k*t,(k+1)*t)
        qrt = qrt_pool.tile([P, nk * t], BF16, name="qrt")

        strips_pool = ctx.enter_context(tc.tile_pool(name="strips", bufs=1))
        # strip j = S_j,(j..nt) = QR_j @ QR^T[:, jP:] as [s(128) x t(width)]
        strips = [
            strips_pool.tile([P, (nt - j) * P], BF16, name=f"strip{j}")
            for j in range(nt)
        ]

        qpool = ctx.enter_context(tc.tile_pool(name="qpool", bufs=2))
        cpool = ctx.enter_context(tc.tile_pool(name="cpool", bufs=2))
        tpool = ctx.enter_context(tc.tile_pool(name="tpool", bufs=1))
        outp = ctx.enter_context(tc.tile_pool(name="outp", bufs=3))

        spsum = ctx.enter_context(tc.tile_pool(name="spsum", bufs=6, space="PSUM"))
        opsum = ctx.enter_context(tc.tile_pool(name="opsum", bufs=2, space="PSUM"))

        qrt3 = qrt.rearrange("p (k tt) -> p k tt", k=nk)
        for w in range(len(bounds) - 1):
            c0, c1 = bounds[w], bounds[w + 1]
            wsz = c1 - c0
            # ---- phase 0: load + RoPE the wave's t-columns ---------------
            # spread each wave's loads over all three DMA paths so no single
            # ring/queue serializes the ramp: Qe half on the sync HWDGE
            # ring, Qo half on gpsimd SWDGE, cos/sin halves on the ACT
            # HWDGE ring.
            q2 = qpool.tile([P, nk, wsz], BF16, tag="q", name=f"q_{w}")
            cs2 = cpool.tile([P, nk, wsz], BF16, tag="cs", name=f"cs_{w}")
            qe2, qo2 = q2[:, 0:half, :], q2[:, half:nk, :]
            ct2, st2 = cs2[:, 0:half, :], cs2[:, half:nk, :]
            nc.sync.dma_start(
                out=qe2,
                in_=qt_d[0 : n // 2, c0:c1].rearrange("(kp p) c -> p kp c", p=P),
            )
            nc.gpsimd.dma_start(
                out=qo2,
                in_=qt_d[n // 2 : n, c0:c1].rearrange("(kp p) c -> p kp c", p=P),
            )
            nc.scalar.dma_start(
                out=ct2,
                in_=cs_d[0 : n // 2, c0:c1].rearrange("(kp p) c -> p kp c", p=P),
            )
            nc.scalar.dma_start(
                out=st2,
                in_=cs_d[n // 2 : n, c0:c1].rearrange("(kp p) c -> p kp c", p=P),
            )
            if w == 0:
                make_upper_triangular(nc, umask, val=1.0, diag=False)
                nc.scalar.dma_start(
                    out=vh.rearrange("p (j dd) -> p j dd", j=nvh),
                    in_=v_d[0 : nvh * P, :].rearrange("(j p) dd -> p j dd", p=P),
                )
            if nt > nvh and w == min(3, len(bounds) - 2):
                # full V, behind this wave's ramp-critical loads on the
                # SWDGE FIFO; first consumed by phase B of this wave
                nc.gpsimd.dma_start(
                    out=vb.rearrange("p (j dd) -> p j dd", j=nt),
                    in_=v_d.rearrange("(j p) dd -> p j dd", p=P),
                )
            # RoPE, batched across all pair chunks (6 big DVE ops per wave):
            #   QRe = Qe*c - Qo*s -> qrt chunks [0, half)
            #   QRo = Qo*c + Qe*s -> qrt chunks [half, nk)
            t1 = tpool.tile([P, half, wsz], BF16, tag="t1", name=f"t1_{w}")
            t2 = tpool.tile([P, half, wsz], BF16, tag="t2", name=f"t2_{w}")
            nc.vector.tensor_mul(t1, qe2, ct2)
            nc.vector.tensor_mul(t2, qo2, st2)
            nc.vector.tensor_sub(qrt3[:, 0:half, c0:c1], t1, t2)
            t3 = tpool.tile([P, half, wsz], BF16, tag="t1", name=f"t3_{w}")
            t4 = tpool.tile([P, half, wsz], BF16, tag="t2", name=f"t4_{w}")
            nc.vector.tensor_mul(t3, qo2, ct2)
            nc.vector.tensor_mul(t4, qe2, st2)
            nc.vector.tensor_add(qrt3[:, half:nk, c0:c1], t3, t4)

            # ---- phase A: score strip tiles landing in wave w ------------
            for j in range(c1 // P):
                lo = max(j * P, c0)
                hi = c1
                width = hi - lo
                ps = spsum.tile([P, width], F32, tag="ps", name=f"ps_{w}_{j}")
                for k in range(nk):
                    nc.tensor.matmul(
                        ps,
                        lhsT=qrt[:, k * t + j * P : k * t + (j + 1) * P],
                        rhs=qrt[:, k * t + lo : k * t + hi],
                        start=(k == 0),
                        stop=(k == nk - 1),
                    )
                l0 = lo - j * P
                if l0 == 0:
                    # diagonal block: strict upper triangle in [s,t]
                    nc.vector.tensor_mul(strips[j][:, 0:P], ps[:, 0:P], umask)
                    if width > P:
                        nc.scalar.copy(strips[j][:, P:width], ps[:, P:width])
                else:
                    nc.scalar.copy(strips[j][:, l0 : l0 + width], ps[:, :width])

            # ---- phase B: outputs for row blocks of wave w ---------------
            for i in range(c0 // P, c1 // P):
                po = opsum.tile([P, d], F32, tag="po", name=f"po_{i}")
                vsrc = vh if i < nvh else vb
                for jj in range(i + 1):
                    nc.tensor.matmul(
                        po,
                        lhsT=strips[jj][:, (i - jj) * P : (i - jj + 1) * P],
                        rhs=vsrc[:, jj * d : (jj + 1) * d],
                        start=(jj == 0),
                        stop=(jj == i),
                    )
                ot = outp.tile([P, d], F32, tag="ot", name=f"ot_{i}")
                nc.scalar.copy(ot, po)
                nc.sync.dma_start(out=out_d[i * P : (i + 1) * P, :], in_=ot)

    nc.compile()
    return nc


def _stage_q(Qc):
    """(t, n) f32 -> (n, t) bf16, de-interleaved: rows [0, n/2) = Qe^T,
    rows [n/2, n) = Qo^T."""
    t, n = Qc.shape
    qp = np.ascontiguousarray(Qc.reshape(t, n // 2, 2).transpose(2, 1, 0))
    return qp.reshape(n, t).astype(ml_dtypes.bfloat16)


def _run(Q, V, trace=False, **trace_kwargs):
    Q = np.asarray(Q, dtype=np.float32)
    V = np.asarray(V, dtype=np.float32)
    b, h, t, n = Q.shape
    d = V.shape[-1]
    ncores = b * h
    nc = _build(t, n, d)
    cs = _rope_tables(t, n)
    in_maps = []
    for core in range(ncores):
        bi, hi = divmod(core, h)
        in_maps.append(
            {
                "qt": _stage_q(Q[bi, hi]),
                "v": np.ascontiguousarray(V[bi, 0].astype(ml_dtypes.bfloat16)),
                "cs": cs,
            }
        )
    res = run_bass_kernel_spmd(
        nc, in_maps, core_ids=list(range(ncores)), trace=trace, **trace_kwargs
    )
    out = np.empty((b, h, t, d), dtype=np.float32)
    for core in range(ncores):
        bi, hi = divmod(core, h)
        out[bi, hi] = res.results[core]["out"]
    return out, res


def kernel(**inputs):
    out, _ = _run(inputs["Q"], inputs["V"], trace=False)
    return out
